# revision 1
# baseline (speedup 1.0000x reference)
"""Single-head causal attention (B=4, S=4096, E=512, D=64) on 8 trn2 cores.

Sharding: 8 cores = 4 batches x 2 query-interleave groups. Core (b, h)
computes output for batch b, query tiles {h, h+2, ..., h+30} (128 rows
each, 16 tiles = 2048 queries). Each core computes K/V for the full
sequence of its batch from x (duplicated across the batch's core pair --
no cross-core collectives).

To keep the SPMD program identical across cores, the host permutes the
key/sequence tiles per core (pair-swap for h=1) so that a core's query
tiles always sit at even SBUF tile slots and the block-causal structure
is slot-identical across cores. Exact causality inside the "diagonal
band" is applied with a data-driven 0/1 mask built on device from
per-core position vectors (qband/kband inputs). The host casts x to
bf16 for the input projections (halves the HBM traffic); scores,
softmax, and the attention accumulation stay in fp32r/fp32.

Per-core program (Tile framework):
  phase B (per 512-key chunk): DMA x^T chunk (bf16), project [K^T|V^T]
    with a single M=128 bf16 matmul per contraction chunk (+bk bias via
    DVE), PE-transpose V^T blocks into [k, d] layout, project Q^T
    half-chunks (+bq).
  phase C (per 512-query chunk c): for key blocks j=0..8c+7 in pairs
    (band pairs first, query-range-trimmed): scoresT_j = K_j Q^T
    (fp32r, contraction d=64), exp via ACT (scale=1/8) into SBUF f32r,
    band mask multiply (DVE), accumulate [V_j|1]^T expT into a psum
    [65, 512] tile = [attnT numerator; softmax denominator], then copy
    to HBM. The division by the denominator and the +bv bias happen on
    the host during the gather.

Pipelining: phase C emission is interleaved with the next chunk's
phase B (emission order = Tile scheduler priority); chunk 0 is emitted
in half-width pieces so the first exp only waits on the first key
chunk's DMA. Per-core cost-model time: ~62 us (input DMA ~12 us, PE
~45 us busy, ACT exp ~38 us busy, all overlapped).
"""

import numpy as np
from contextlib import ExitStack

import concourse.mybir as mybir
import concourse.tile as tile
from concourse import bacc
from concourse.bass_utils import run_bass_kernel_spmd
from concourse.masks import make_identity

F32 = mybir.dt.float32
F32R = mybir.dt.float32r
BF16 = mybir.dt.bfloat16
AF = mybir.ActivationFunctionType
OP = mybir.AluOpType

B, S, E, D = 4, 4096, 512, 64
P = 128
EO = E // P           # 4 contraction chunks of 128
NT = S // P           # 32 key tiles
KC = S // 512         # 8 key chunks
QC = (S // 2) // 512  # 4 query chunks per core
N_CORES = 8

_CACHE: dict = {}


def _build():
    nc = bacc.Bacc(
        "TRN2", target_bir_lowering=False, debug=False, num_devices=N_CORES
    )
    xkT = nc.dram_tensor("xkT", [E, S], BF16, kind="ExternalInput").ap()
    w3 = nc.dram_tensor("w3", [E, 3 * D], BF16, kind="ExternalInput").ap()
    bias2 = nc.dram_tensor("bias2", [P, 2], F32, kind="ExternalInput").ap()
    qband = nc.dram_tensor("qband", [P, 512], F32, kind="ExternalInput").ap()
    kband = nc.dram_tensor("kband", [P, 8], F32, kind="ExternalInput").ap()
    # rows 0:64 attnT numerator, row 64 softmax denominator
    outT = nc.dram_tensor("outT", [D + 1, S // 2], F32, kind="ExternalOutput").ap()

    with tile.TileContext(nc) as tc, ExitStack() as ctx:
        sb_const = ctx.enter_context(tc.tile_pool(name="const", bufs=1))
        sb_kv = ctx.enter_context(tc.tile_pool(name="kv", bufs=1))
        sb_xk = ctx.enter_context(tc.tile_pool(name="xk", bufs=6))
        sb_exp = ctx.enter_context(tc.tile_pool(name="exp", bufs=5))
        ps_misc = ctx.enter_context(tc.tile_pool(name="psm", bufs=2, space="PSUM"))
        ps_sc = ctx.enter_context(tc.tile_pool(name="pssc", bufs=2, space="PSUM"))
        ps_at = ctx.enter_context(tc.tile_pool(name="psat", bufs=1, space="PSUM"))
        ps_q = ctx.enter_context(tc.tile_pool(name="psq", bufs=1, space="PSUM"))

        # ---------------- constants ----------------
        w3t = sb_const.tile([P, EO, 3 * D], BF16)
        nc.sync.dma_start(
            w3t[:], w3.rearrange("(eo p) d -> p eo d", p=P)
        )
        b2 = sb_const.tile([P, 2], F32)
        nc.sync.dma_start(b2[:], bias2)
        qb = sb_const.tile([P, 512], F32)
        kb = sb_const.tile([P, 8], F32)
        onesF = sb_const.tile([P, D], F32)
        nc.gpsimd.memset(onesF[:], 1.0)
        identF = sb_const.tile([P, P], F32)
        make_identity(nc, identF[:])
        ident = sb_const.tile([P, P], F32R)
        nc.vector.tensor_copy(ident[:], identF[:])
        # 0/1 causal band masks, restricted to the slots that actually
        # need masking: band pair m covers query cols [qs(m), qs(m)+w(m))
        # with qs = 128*m (m<3) else 256, w = 128 (m<3) else 256.
        bmask = sb_const.tile([P, 8, P], F32R)

        def build_masks():
            nc.sync.dma_start(qb[:], qband)
            nc.sync.dma_start(kb[:], kband)
            for jl in range(8):
                m = jl // 2
                qc0 = 128 * m
                nc.vector.tensor_tensor(
                    out=bmask[:, jl, :],
                    in0=qb[:, qc0 : qc0 + P],
                    in1=kb[:, jl : jl + 1].to_broadcast((P, P)),
                    op=OP.is_ge,
                )

        # ---------------- persistent state ----------------
        # kv_all rows 0:64 = biased K^T, rows 64:128 = V^T
        kv_all = sb_kv.tile([P, S], F32R)
        qts = sb_kv.tile([D, S // 2], F32R)
        # V blocks in [k, d] layout plus a ones column for the denominator
        vo = sb_kv.tile([P, NT, D + 1], F32R)
        nc.vector.tensor_copy(vo[:, :, D], onesF[:, 0:NT])

        def phase_b(kc):
            xk = sb_xk.tile([P, EO, 512], BF16, tag="xk", name=f"xk{kc}")
            src = xkT[:, kc * 512 : (kc + 1) * 512].rearrange(
                "(eo p) k -> p eo k", p=P
            )
            if kc < 2:
                # per-eo DMAs so the first matmuls start at 1/4 of the load
                for eo in range(EO):
                    nc.sync.dma_start(xk[:, eo, :], src[:, eo, :])
            else:
                nc.sync.dma_start(xk[:], src)
            # [K|V] projection: single M=128 stationary operand; bias
            # col 0 holds bk on rows 0:64 and zeros on rows 64:128
            pkv = ps_misc.tile([P, 512], F32, tag="ps", name=f"pkv{kc}")
            for eo in range(EO):
                nc.tensor.matmul(
                    pkv[:],
                    w3t[:, eo, D : 3 * D],
                    xk[:, eo, :],
                    start=(eo == 0),
                    stop=(eo == EO - 1),
                )
            nc.vector.tensor_tensor(
                out=kv_all[:, kc * 512 : (kc + 1) * 512],
                in0=pkv[:],
                in1=b2[:, 0:1].to_broadcast((P, 512)),
                op=OP.add,
            )
            return xk


        def phase_b_tr(kc):
            # V^T -> V transposes (4 key blocks per chunk) into one psum
            # bank, then a single strided DVE copy into vo; V bias is
            # applied on the host after the division
            pt = ps_misc.tile([P, 512], F32, tag="ps", name=f"pt{kc}")
            for bb in range(4):
                j = 4 * kc + bb
                nc.tensor.transpose(
                    pt[:, bb * D : (bb + 1) * D].bitcast(F32R),
                    kv_all[D:P, j * P : (j + 1) * P],
                    ident[D:P, D:P],
                )
            nc.vector.tensor_copy(
                vo[:, 4 * kc : 4 * kc + 4, 0:D],
                pt[:, 0 : 4 * D].rearrange("p (b d) -> p b d", d=D),
            )

        pq_tiles = {}

        def phase_b_q_half(c, xk_h, half):
            # Q^T half-chunk from the even local tiles of one key chunk
            if half == 0:
                pq_tiles[c] = ps_q.tile([P, 512], F32, tag="pq", name=f"pq{c}")
            pq = pq_tiles[c]
            for eo in range(EO):
                rhs = xk_h[:, eo, :].rearrange(
                    "p (t2 two x) -> p t2 two x", two=2, x=P
                )[:, :, 0, :]
                nc.tensor.matmul(
                    pq[0:D, half * 256 : (half + 1) * 256],
                    w3t[:, eo, 0:D],
                    rhs,
                    start=(eo == 0),
                    stop=(eo == EO - 1),
                )
            nc.vector.tensor_tensor(
                out=qts[:, c * 512 + half * 256 : c * 512 + (half + 1) * 256],
                in0=pq[0:D, half * 256 : (half + 1) * 256],
                in1=b2[0:D, 1:2].to_broadcast((D, 256)),
                op=OP.add,
            )

        def phase_b_q(c, xk_a, xk_b):
            phase_b_q_half(c, xk_a, 0)
            phase_b_q_half(c, xk_b, 1)

        def phase_c(c, inject=None):
            pat = ps_at.tile([D + 1, 512], F32, tag="at", name=f"at{c}")
            npair = 4 * c + 4
            # band pairs first: they depend on the same key chunks as this
            # chunk's Q projection, and ending on a mask-free full-width
            # pair shortens the critical tail
            order = list(range(4 * c, npair)) + list(range(0, 4 * c))
            for idx, p2 in enumerate(order):
                if inject and idx in inject:
                    for fn in inject[idx]:
                        fn()
                j0, j1 = 2 * p2, 2 * p2 + 1
                m = p2 - 4 * c  # band pair index, >= 0 inside the band
                qs = 0 if m < 0 else min(128 * m, 256)  # matmul region
                qe = 0 if m < 0 else 128 * m            # exp/mask/AV region
                psc = ps_sc.tile([P, 1024], F32, tag="sc", name=f"sc{c}_{p2}")
                # row-tiled pair: j0 on array rows 0:64, j1 on rows 64:128
                nc.tensor.matmul(
                    psc[:, qs:512],
                    kv_all[0:D, j0 * P : (j0 + 1) * P],
                    qts[:, c * 512 + qs : (c + 1) * 512],
                    start=True,
                    stop=True,
                    tile_position=(0, 0),
                )
                nc.tensor.matmul(
                    psc[:, 512 + qs : 1024],
                    kv_all[0:D, j1 * P : (j1 + 1) * P],
                    qts[:, c * 512 + qs : (c + 1) * 512],
                    start=True,
                    stop=True,
                )
                eT = sb_exp.tile([P, 1024], F32R, tag="eT", name=f"eT{c}_{p2}")
                psc_v = psc[:].rearrange("p (two x) -> p two x", x=512)
                eT_v = eT[:].rearrange("p (two x) -> p two x", x=512)
                nc.scalar.activation(
                    eT_v[:, :, qe:512], psc_v[:, :, qe:512], AF.Exp, scale=0.125
                )
                if m >= 0:
                    nc.vector.tensor_mul(
                        eT_v[:, :, qe : qe + P],
                        eT_v[:, :, qe : qe + P],
                        bmask[:, 2 * m : 2 * m + 2, :],
                    )
                nc.tensor.matmul(
                    pat[:, qe:512],
                    vo[:, j0, :],
                    eT[:, qe:512],
                    start=(idx == 0),
                    stop=False,
                )
                nc.tensor.matmul(
                    pat[:, qe:512],
                    vo[:, j1, :],
                    eT[:, 512 + qe : 1024],
                    start=False,
                    stop=(idx == npair - 1),
                )
            osb = sb_exp.tile([D + 1, 512], F32, tag="osb", name=f"osb{c}")
            nc.vector.tensor_copy(osb[:], pat[:])
            nc.sync.dma_start(outT[:, c * 512 : (c + 1) * 512], osb[:])

        # emission order = scheduler priority: each chunk pair's KV
        # projection + Q projection + transposes are emitted BEFORE the
        # previous chunk's phase C so the next chunk's inputs are ready
        # the moment the ACT pipeline drains
        xk_tiles = {}

        def mk(fn, *args):
            return lambda: fn(*args)

        def emit_b(kc):
            xk_tiles[kc] = phase_b(kc)

        def emit_q(c):
            phase_b_q(c, xk_tiles[2 * c], xk_tiles[2 * c + 1])

        def phase_c0_piece(pat0, m, h):
            # chunk-0 pair m restricted to query cols [a, b)
            qe = 128 * m
            a, b = max(qe, 256 * h), 256 * h + 256
            w = b - a
            if w <= 0:
                return
            j0, j1 = 2 * m, 2 * m + 1
            psc = ps_sc.tile([P, 1024], F32, tag="sc", name=f"s0_{m}_{h}")
            for ji, j in ((0, j0), (1, j1)):
                nc.tensor.matmul(
                    psc[:, 512 * ji : 512 * ji + w],
                    kv_all[0:D, j * P : (j + 1) * P],
                    qts[:, a:b],
                    start=True,
                    stop=True,
                )
            eT = sb_exp.tile([P, 1024], F32R, tag="eT", name=f"e0_{m}_{h}")
            psc_v = psc[:].rearrange("p (two x) -> p two x", x=512)
            eT_v = eT[:].rearrange("p (two x) -> p two x", x=512)
            nc.scalar.activation(
                eT_v[:, :, 0:w], psc_v[:, :, 0:w], AF.Exp, scale=0.125
            )
            if a <= qe < b:
                lo = qe - a
                nc.vector.tensor_mul(
                    eT_v[:, :, lo : lo + P],
                    eT_v[:, :, lo : lo + P],
                    bmask[:, 2 * m : 2 * m + 2, :],
                )
            nc.tensor.matmul(
                pat0[:, a:b], vo[:, j0, :], eT[:, 0:w],
                start=(m == 0), stop=False,
            )
            nc.tensor.matmul(
                pat0[:, a:b], vo[:, j1, :], eT[:, 512 : 512 + w],
                start=False, stop=(m == 3 and h == 1),
            )

        if True:  # chunk-0 split head schedule
            emit_b(0)
            build_masks()
            phase_b_q_half(0, xk_tiles[0], 0)
            phase_b_tr(0)
            pat0 = ps_at.tile([D + 1, 512], F32, tag="at", name="at0")
            phase_c0_piece(pat0, 0, 0)
            phase_c0_piece(pat0, 1, 0)
            emit_b(1)
            phase_b_q_half(0, xk_tiles[1], 1)
            phase_b_tr(1)
            phase_c0_piece(pat0, 0, 1)
            emit_b(2)
            phase_c0_piece(pat0, 1, 1)
            emit_b(3)
            phase_c0_piece(pat0, 2, 1)
            emit_q(1)
            phase_c0_piece(pat0, 3, 1)
            phase_b_tr(2)
            phase_b_tr(3)
            osb0 = sb_exp.tile([D + 1, 512], F32, tag="osb", name="osb0")
            nc.vector.tensor_copy(osb0[:], pat0[:])
            nc.sync.dma_start(outT[:, 0:512], osb0[:])
            c_start = 1
        else:
            emit_b(0)
            build_masks()
            emit_b(1)
            emit_q(0)
            phase_b_tr(0)
            phase_b_tr(1)
            c_start = 0
        inj_at = {0: [1, 2, 3, 3], 1: [4, 6, 7, 7], 2: [5, 8, 11, 11]}
        for c in range(c_start, QC):
            if c < QC - 1:
                cn = c + 1
                pts = inj_at[c]
                items = [
                    [mk(emit_b, 2 * cn)],
                    [mk(emit_b, 2 * cn + 1)],
                    [mk(emit_q, cn)],
                    [mk(phase_b_tr, 2 * cn), mk(phase_b_tr, 2 * cn + 1)],
                ]
                inject = {}
                for pt, fns in zip(pts, items):
                    inject.setdefault(pt, []).extend(fns)
            else:
                inject = None
            phase_c(c, inject)

    nc.compile()
    return nc


def _stage_inputs(x, Wq, bq, Wk, bk, Wv, bv):
    """Build the 8 per-core input dicts."""
    import ml_dtypes

    x = np.asarray(x, dtype=np.float32)
    w3 = np.concatenate(
        [np.asarray(Wq), np.asarray(Wk), np.asarray(Wv)], axis=1
    ).astype(ml_dtypes.bfloat16)
    bias2 = np.zeros((P, 2), dtype=np.float32)
    bias2[0:D, 0] = np.asarray(bk, dtype=np.float32)  # K rows 0:64
    bias2[0:D, 1] = np.asarray(bq, dtype=np.float32)  # Q rows 0:64
    # bv is applied on the host during the gather

    qv = np.arange(512)
    in_maps = []
    for core in range(N_CORES):
        b, h = divmod(core, 2)
        g = np.arange(NT)
        if h == 1:
            g = g ^ 1  # pair-swap so query tiles land on even slots
        xb = x[b].reshape(NT, P, E)[g]  # [32,128,512]
        xkT_c = np.ascontiguousarray(
            xb.reshape(S, E).T.astype(ml_dtypes.bfloat16)
        )  # [512, 4096] bf16
        qpos = (P * (2 * (qv // P) + h) + (qv % P)).astype(np.float32)
        qband = np.ascontiguousarray(np.broadcast_to(qpos, (P, 512)))
        kk = np.arange(P)
        jl = np.arange(8)
        kband = (P * (jl[None, :] ^ h) + kk[:, None]).astype(np.float32)
        in_maps.append(
            {
                "xkT": xkT_c,
                "w3": w3,
                "bias2": bias2,
                "qband": qband,
                "kband": np.ascontiguousarray(kband),
            }
        )
    return in_maps


def _gather_output(results, bv):
    """Merge 8 per-core outT [65, 2048] into the full [B, S, D] output."""
    out = np.empty((B, S, D), dtype=np.float32)
    bv = np.asarray(bv, dtype=np.float32)
    tg = np.array([8 * c + 2 * si for c in range(QC) for si in range(4)])
    for core in range(N_CORES):
        b, h = divmod(core, 2)
        ot = results[core]["outT"]  # [65, 2048]
        attn = ot[0:D] / ot[D : D + 1] + bv[:, None]  # denom + V bias
        blocks = attn.T.reshape(16, P, D)  # [(c,si), r, d]
        out.reshape(B, NT, P, D)[b, tg + h] = blocks
    return out


def kernel(x, Wq, bq, Wk, bk, Wv, bv):
    if "nc" not in _CACHE:
        _CACHE["nc"] = _build()
    nc = _CACHE["nc"]
    in_maps = _stage_inputs(x, Wq, bq, Wk, bk, Wv, bv)
    res = run_bass_kernel_spmd(nc, in_maps, core_ids=list(range(N_CORES)))
    return _gather_output(res.results, bv)



# revision 6
# speedup vs baseline: 1.1604x; 1.1604x over previous
"""Single-head causal attention (B=4, S=4096, E=512, D=64) on 8 trn2 cores.

Sharding: 8 cores = 4 batches x 2 query-interleave groups. Core (b, h)
computes output for batch b, query tiles {h, h+2, ..., h+30} (128 rows
each, 16 tiles = 2048 queries). Each core computes K/V for the full
sequence of its batch from x (duplicated across the batch's core pair --
no cross-core collectives). The host permutes key tiles per core
(pair-swap for h=1) so query tiles sit at even local slots and the
block-causal structure is slot-identical across cores.

Numerics: x and the (16x-scaled) weights are fp8e4; QKV projections run
as fp8 DoubleRow matmuls (2 contraction k-tiles streamed per cycle).
Q^T/K^T are requantized to fp8 with a zero second k-tile plane so the
scores matmuls also run in DoubleRow mode at half cost. The causal mask
inside the diagonal band is applied by accumulating a host-provided
triangular -51200 matrix into the score psum via a bf16 matmul
(replaces per-pair DVE mask multiplies). Softmax exp runs with a
uniform exp(s/8 - 1) shift (cancels in softmax; keeps exp < fp8e4 max):
most pairs on ACT (fp8 output), a subset of full-width pairs on DVE via
a Schraudolph bit-trick (u16 = A*s + B bitcast to bf16 ~= exp). The
attention*V accumulation uses one DoubleRow fp8 matmul per block pair
([V_j0|1; V_j1|1] stationary, exp pair moving) into a [65, 512] psum
(numerator rows 0:64, denominator row 64); Schraudolph pairs use two
bf16 matmuls against a bf16 V copy. Host divides, unscales, adds bv.
"""

import numpy as np
from contextlib import ExitStack

import concourse.mybir as mybir
import concourse.tile as tile
from concourse import bacc
from concourse.bass_utils import run_bass_kernel_spmd
from concourse.masks import make_identity

F32 = mybir.dt.float32
BF16 = mybir.dt.bfloat16
FP8 = mybir.dt.float8e4
U16 = mybir.dt.uint16
AF = mybir.ActivationFunctionType
OP = mybir.AluOpType
DR = mybir.MatmulPerfMode.DoubleRow

B, S, E, D = 4, 4096, 512, 64
P = 128
NT = S // P           # 32 key tiles
KC = S // 512         # 8 key chunks
QC = (S // 2) // 512  # 4 query chunks per core
N_CORES = 8

WSCALE = 16.0         # host scales W (and biases) by 16 for fp8 range
MASKVAL = -51200.0    # -25 * 2048 in score units -> exp ~ e^-26
EXP_SCALE = 1.0 / 2048.0   # score -> exponent (1/8 softmax * 1/256 Wscale)
EXP_BIAS = -1.0            # uniform shift; cancels in softmax
# Schraudolph u16/bf16 exp: u16 = SCH_A * score + SCH_B, bitcast bf16
SCH_A = 128.0 * 1.4426950408889634 * EXP_SCALE
SCH_B = 16256.0 - 128.0 * 1.4426950408889634 - 5.5

# pairs computed on DVE (Schraudolph) per query chunk: full-width
# non-band pairs with small p2 (their key blocks get a bf16 V copy)
DVE_PAIRS = {0: [], 1: [0, 1], 2: [0, 1, 2], 3: [0, 1, 2, 3]}
NBF = 4  # bf16 V copies for pairs 0..NBF-1 (blocks 0..2*NBF-1)

_CACHE: dict = {}


def _build():
    nc = bacc.Bacc(
        "TRN2", target_bir_lowering=False, debug=False, num_devices=N_CORES
    )
    xdr = nc.dram_tensor("xdr", [KC, P, 2, 2, 512], FP8, kind="ExternalInput").ap()
    xq = nc.dram_tensor("xq", [QC, P, 2, 2, 512], FP8, kind="ExternalInput").ap()
    w3 = nc.dram_tensor("w3", [P, 2, 2, 192], FP8, kind="ExternalInput").ap()
    bias2 = nc.dram_tensor("bias2", [P, 2], F32, kind="ExternalInput").ap()
    mtri = nc.dram_tensor("mtri", [P, 2, P], BF16, kind="ExternalInput").ap()
    # rows 0:64 attnT numerator (16x), row 64 softmax denominator
    outT = nc.dram_tensor("outT", [D + 1, S // 2], F32, kind="ExternalOutput").ap()

    with tile.TileContext(nc) as tc, ExitStack() as ctx:
        sb_const = ctx.enter_context(tc.tile_pool(name="const", bufs=1))
        sb_kv = ctx.enter_context(tc.tile_pool(name="kv", bufs=1))
        sb_xk = ctx.enter_context(tc.tile_pool(name="xk", bufs=6))
        sb_exp = ctx.enter_context(tc.tile_pool(name="exp", bufs=5))
        sb_u16 = ctx.enter_context(tc.tile_pool(name="u16", bufs=3))
        ps_misc = ctx.enter_context(tc.tile_pool(name="psm", bufs=2, space="PSUM"))
        ps_sc = ctx.enter_context(tc.tile_pool(name="pssc", bufs=2, space="PSUM"))
        ps_at = ctx.enter_context(tc.tile_pool(name="psat", bufs=1, space="PSUM"))
        ps_q = ctx.enter_context(tc.tile_pool(name="psq", bufs=1, space="PSUM"))

        # ---------------- constants ----------------
        w3t = sb_const.tile([P, 2, 2, 192], FP8)
        nc.sync.dma_start(w3t[:], w3)
        b2 = sb_const.tile([P, 2], F32)
        nc.sync.dma_start(b2[:], bias2)
        mt = sb_const.tile([P, 2, P], BF16)
        nc.sync.dma_start(mt[:], mtri)
        identF = sb_const.tile([P, P], F32)
        make_identity(nc, identF[:])
        # [64,64] fp8 identity at partition base 64 (transpose rhs must
        # share the stationary operand's base partition)
        ident8 = sb_const.tile([P, D], FP8)
        nc.vector.tensor_copy(ident8[D:P, :], identF[0:D, 0:D])
        identM = sb_const.tile([P, P], BF16)
        nc.vector.tensor_copy(identM[:], identF[:])
        bconst = sb_const.tile([P, 1], F32)
        nc.gpsimd.memset(bconst[:], float(SCH_B))
        ebias = sb_const.tile([P, 1], F32)
        nc.gpsimd.memset(ebias[:], float(EXP_BIAS))

        # ---------------- persistent state ----------------
        # K^T (fp8, 16x, +16bk) on partitions 0:64 / V^T (fp8, 16x) on
        # 64:128; per block j: [p, j, t, col] with t=1 zero (DoubleRow)
        kvdr = sb_kv.tile([P, NT, 2, P], FP8)
        # Q^T fp8 per query chunk: [p(d), c, t, q] with t=1 zero
        qdr = sb_kv.tile([D, QC, 2, 512], FP8)
        # V in [k, d] layout + ones column, paired for DoubleRow AV
        vo = sb_kv.tile([P, NT // 2, 2, D + 1], FP8)
        vobf = sb_kv.tile([P, NBF, 2, D + 1], BF16)
        nc.gpsimd.memset(vo[:, :, :, D], 1.0)
        nc.gpsimd.memset(vobf[:, :, :, D], 1.0)

        def phase_b(kc):
            xk = sb_xk.tile([P, 2, 2, 512], FP8, tag="xk", name=f"xk{kc}")
            if kc < 2:
                for i in range(2):
                    nc.sync.dma_start(xk[:, i], xdr[kc, :, i])
            else:
                nc.sync.dma_start(xk[:], xdr[kc])
            # zero the DoubleRow t=1 planes for this chunk's K/V blocks
            nc.gpsimd.memset(kvdr[:, 4 * kc : 4 * kc + 4, 1, :], 0.0)
            # [K|V] projection: 4 fp8 DoubleRow matmuls (contraction 128
            # each as 2 k-tiles of 64)
            pkv = ps_misc.tile([P, 512], F32, tag="ps", name=f"pkv{kc}")
            for mi, (hb, i) in enumerate(((0, 0), (0, 1), (1, 0), (1, 1))):
                nc.tensor.matmul(
                    pkv[:],
                    w3t[64 * hb : 64 * hb + 64, i, :, 0:128],
                    xk[64 * hb : 64 * hb + 64, i, :, :],
                    start=(mi == 0),
                    stop=(mi == 3),
                    perf_mode=DR,
                )
            # bias col 0 = 16bk on rows 0:64, zeros on 64:128; fp8 out
            nc.vector.tensor_tensor(
                out=kvdr[:, 4 * kc : 4 * kc + 4, 0, :],
                in0=pkv[:].rearrange("p (b c) -> p b c", c=P),
                in1=b2[:, 0:1].to_broadcast((P, 4, P)),
                op=OP.add,
            )
            return xk

        def phase_b_q(c):
            xqt = sb_xk.tile([P, 2, 2, 512], FP8, tag="xk", name=f"xq{c}")
            nc.sync.dma_start(xqt[:], xq[c])
            nc.gpsimd.memset(qdr[:, c, 1, :], 0.0)
            pq = ps_q.tile([D, 512], F32, tag="pq", name=f"pq{c}")
            for mi, (hb, i) in enumerate(((0, 0), (0, 1), (1, 0), (1, 1))):
                nc.tensor.matmul(
                    pq[:],
                    w3t[64 * hb : 64 * hb + 64, i, :, 128:192],
                    xqt[64 * hb : 64 * hb + 64, i, :, :],
                    start=(mi == 0),
                    stop=(mi == 3),
                    perf_mode=DR,
                )
            nc.vector.tensor_tensor(
                out=qdr[:, c, 0, :],
                in0=pq[:],
                in1=b2[0:D, 1:2].to_broadcast((D, 512)),
                op=OP.add,
            )

        def phase_b_tr(grp):
            # V^T -> V transposes for blocks 8g..8g+7 (fp8: out elem
            # step 2), then one strided DVE copy into vo (+ bf16 copy
            # for the Schraudolph pairs' blocks)
            pt = ps_misc.tile([P, 8, D, 2], FP8, tag="ps", name=f"pt{grp}")
            for bb in range(8):
                j = 8 * grp + bb
                nc.tensor.transpose(
                    pt[:, bb, :, 0],
                    kvdr[D:P, j, 0, :],
                    ident8[D:P, :],
                )
            nc.vector.tensor_copy(
                vo[:, 4 * grp : 4 * grp + 4, :, 0:D],
                pt[:, :, :, 0].rearrange("p (pr two) d -> p pr two d", two=2),
            )
            if 4 * grp < NBF:
                nc.vector.tensor_copy(
                    vobf[:, 4 * grp : 4 * grp + 4, :, 0:D],
                    pt[:, :, :, 0].rearrange("p (pr two) d -> p pr two d", two=2),
                )

        def phase_c(c, inject=None):
            pat = ps_at.tile([D + 1, 512], F32, tag="at", name=f"at{c}")
            npair = 4 * c + 4
            dve_set = set(DVE_PAIRS[c])
            # band pairs first (freshest K), then ACT non-band, then DVE
            # non-band last (overlaps ACT's start of the next chunk)
            order = (
                list(range(4 * c, npair))
                + [p for p in range(0, 4 * c) if p not in dve_set]
                + [p for p in range(0, 4 * c) if p in dve_set]
            )
            first_av = order[0]
            last_av = order[-1]
            for idx, p2 in enumerate(order):
                if inject and idx in inject:
                    for fn in inject[idx]:
                        fn()
                j0, j1 = 2 * p2, 2 * p2 + 1
                m = p2 - 4 * c  # band pair index, >= 0 inside the band
                qs = 0 if m < 0 else min(128 * m, 256)  # matmul region
                qe = 0 if m < 0 else 128 * m            # exp/AV region
                psc = ps_sc.tile([P, 1024], F32, tag="sc", name=f"sc{c}_{p2}")
                for ji, j in ((0, j0), (1, j1)):
                    nc.tensor.matmul(
                        psc[:, 512 * ji + qs : 512 * ji + 512],
                        kvdr[0:D, j, :, :],
                        qdr[:, c, :, qs:512],
                        start=True,
                        stop=(m < 0),
                        perf_mode=DR,
                    )
                if m >= 0:
                    # causal band mask: accumulate tri/const -51200 via
                    # bf16 matmul into the diagonal 128-col region
                    for ji in range(2):
                        nc.tensor.matmul(
                            psc[:, 512 * ji + qe : 512 * ji + qe + P],
                            mt[:, ji, :],
                            identM[:],
                            start=False,
                            stop=True,
                        )
                psc_v = psc[:].rearrange("p (two x) -> p two x", x=512)
                if p2 in dve_set:
                    u16 = sb_u16.tile([P, 1024], U16, tag="u16", name=f"u{c}_{p2}")
                    nc.vector.scalar_tensor_tensor(
                        out=u16[:],
                        in0=psc[:],
                        scalar=float(SCH_A),
                        in1=bconst[:].to_broadcast((P, 1024)),
                        op0=OP.mult,
                        op1=OP.add,
                    )
                    ebf = u16[:].bitcast(BF16).rearrange("p (two x) -> p two x", x=512)
                    for ji in range(2):
                        nc.tensor.matmul(
                            pat[:, 0:512],
                            vobf[:, p2, ji, :],
                            ebf[:, ji, :],
                            start=(idx == 0 and ji == 0 and p2 == first_av),
                            stop=(p2 == last_av and ji == 1),
                        )
                else:
                    eT = sb_exp.tile([P, 1024], FP8, tag="eT", name=f"eT{c}_{p2}")
                    eT_v = eT[:].rearrange("p (two x) -> p two x", x=512)
                    nc.scalar.activation(
                        eT_v[:, :, qe:512],
                        psc_v[:, :, qe:512],
                        AF.Exp,
                        scale=float(EXP_SCALE),
                        bias=ebias[:],
                    )
                    nc.tensor.matmul(
                        pat[:, qe:512],
                        vo[:, p2, :, :],
                        eT_v[:, :, qe:512],
                        start=(p2 == first_av),
                        stop=(p2 == last_av),
                        perf_mode=DR,
                    )
            osb = sb_exp.tile([D + 1, 512], F32, tag="osb", name=f"osb{c}")
            nc.vector.tensor_copy(osb[:], pat[:])
            nc.sync.dma_start(outT[:, c * 512 : (c + 1) * 512], osb[:])

        def mk(fn, *args):
            return lambda: fn(*args)

        # ---------------- schedule ----------------
        phase_b(0)
        phase_b_q(0)
        phase_b(1)
        phase_b_tr(0)
        inj_at = {
            0: {1: [mk(phase_b, 2)], 2: [mk(phase_b, 3)],
                3: [mk(phase_b_q, 1), mk(phase_b_tr, 1)]},
            1: {2: [mk(phase_b, 4)], 4: [mk(phase_b, 5)],
                6: [mk(phase_b_q, 2)], 7: [mk(phase_b_tr, 2)]},
            2: {3: [mk(phase_b, 6)], 6: [mk(phase_b, 7)],
                9: [mk(phase_b_q, 3)], 11: [mk(phase_b_tr, 3)]},
            3: None,
        }
        for c in range(QC):
            phase_c(c, inj_at[c])

    nc.compile()
    return nc


def _stage_inputs(x, Wq, bq, Wk, bk, Wv, bv):
    """Build the 8 per-core input dicts."""
    import ml_dtypes

    NP8 = ml_dtypes.float8_e4m3

    x = np.asarray(x, dtype=np.float32)
    # cols 0:64 = 16*Wk, 64:128 = 16*Wv, 128:192 = 16*Wq
    w3 = np.concatenate(
        [np.asarray(Wk), np.asarray(Wv), np.asarray(Wq)], axis=1
    ).astype(np.float32) * WSCALE

    def dr_fold(mat):
        # [512, C] -> [128, 2, 2, C]: partition = half*64+p, dims (i, t)
        C = mat.shape[1]
        return np.ascontiguousarray(
            mat.reshape(2, 2, 2, 64, C).transpose(0, 3, 1, 2, 4)
            .reshape(P, 2, 2, C)
        )

    w3dr = dr_fold(w3).astype(NP8)

    bias2 = np.zeros((P, 2), dtype=np.float32)
    bias2[0:D, 0] = np.asarray(bk, dtype=np.float32) * WSCALE
    bias2[0:D, 1] = np.asarray(bq, dtype=np.float32) * WSCALE

    in_maps = []
    for core in range(N_CORES):
        b, h = divmod(core, 2)
        g = np.arange(NT)
        if h == 1:
            g = g ^ 1  # pair-swap so query tiles land on even slots
        xb = x[b].reshape(NT, P, E)[g]          # [32,128,512]
        xT = xb.reshape(S, E).T                  # [512, 4096]
        xdr_full = dr_fold(xT).astype(NP8)       # [128, 2, 2, 4096]
        xdr_c = np.ascontiguousarray(
            xdr_full.reshape(P, 2, 2, KC, 512).transpose(3, 0, 1, 2, 4)
        )                                        # [8, 128, 2, 2, 512]
        # queries = even local tiles: chunk c covers local tiles
        # {8c, 8c+2, 8c+4, 8c+6}
        xq_tiles = xdr_full.reshape(P, 2, 2, NT, P)[:, :, :, 0::2, :]
        xq_c = np.ascontiguousarray(
            xq_tiles.reshape(P, 2, 2, QC, 512).transpose(3, 0, 1, 2, 4)
        )                                        # [4, 128, 2, 2, 512]
        # band mask: plane 0 = strict upper tri (key > query in same
        # tile), plane 1 = const (j1 block fully masked for h=0,
        # visible for h=1)
        tri = np.where(
            np.arange(P)[None, :] > np.arange(P)[:, None], MASKVAL, 0.0
        ).astype(np.float32)
        const = np.full((P, P), MASKVAL if h == 0 else 0.0, dtype=np.float32)
        mtri = np.ascontiguousarray(
            np.stack([tri, const], axis=1)
        ).astype(ml_dtypes.bfloat16)
        in_maps.append(
            {
                "xdr": xdr_c,
                "xq": xq_c,
                "w3": w3dr,
                "bias2": bias2,
                "mtri": mtri,
            }
        )
    return in_maps


def _gather_output(results, bv):
    """Merge 8 per-core outT [65, 2048] into the full [B, S, D] output."""
    out = np.empty((B, S, D), dtype=np.float32)
    bv = np.asarray(bv, dtype=np.float32)
    tg = np.array([8 * c + 2 * si for c in range(QC) for si in range(4)])
    for core in range(N_CORES):
        b, h = divmod(core, 2)
        ot = results[core]["outT"]  # [65, 2048]
        attn = ot[0:D] / ot[D : D + 1] / WSCALE + bv[:, None]
        blocks = attn.T.reshape(16, P, D)  # [(c,si), r, d]
        out.reshape(B, NT, P, D)[b, tg + h] = blocks
    return out


def kernel(x, Wq, bq, Wk, bk, Wv, bv):
    if "nc" not in _CACHE:
        _CACHE["nc"] = _build()
    nc = _CACHE["nc"]
    in_maps = _stage_inputs(x, Wq, bq, Wk, bk, Wv, bv)
    res = run_bass_kernel_spmd(nc, in_maps, core_ids=list(range(N_CORES)))
    return _gather_output(res.results, bv)
